# revision 1
# baseline (speedup 1.0000x reference)
"""Trainium2 Bass kernel for full-embed-dim self-attention + residual LayerNorm.

Problem: B=4, S=2048, D=1024 fp32.
  q/k/v = x@w{q,k,v}+b; scores = q@k^T/sqrt(D); attn = softmax(scores)@v;
  out = LN(x + attn@wo + bo) * gamma + beta.

Sharding: 8 cores = 4 batches x 2 query-halves (1024 queries each). Each
core computes K/V projections only for its own 1024 keys, then a pair-wise
AllGather ([0,1],[2,3],...) exchanges the halves so each core attends over
the batch's full 2048-key sequence.

Per-core dataflow (d-on-partitions "transposed" layout throughout):
  QT[d,q]   = wq^T @ xq^T            (host ships x^T for the core's half)
  KT[d,k_own] = wk^T @ xq^T (+bk per-partition)   --> AllGather --> KT full
  V[k_own,d]  = xq^T^T @ wv                       --> AllGather --> V full
  ST[k,q]   = KT^T @ QT              (contracted over d in PSUM; bq is
               added to QT per-partition, bk to KT, so ST is the true logits)
  PT[k,q]   = exp(ST/sqrt(D))        (logits are O(5) std-1, so exp needs
               no max-subtraction in fp32)
  denom[q]  = ones^T @ PT            (PE column-sum)
  AT[d,q]   = V^T @ PT
  O[q,e]    = AT^T @ wo, then O/denom[q] + xq_aug (host adds bo+bv@wo), LN.

Matmul inputs are bf16 (1 PE cycle/row vs 4 for fp32; fp32 PSUM accum).
"""

import numpy as np
import ml_dtypes

import concourse.bass as bass
import concourse.mybir as mybir
import concourse.tile as tile
from concourse import bacc

F32 = mybir.dt.float32
BF16 = mybir.dt.bfloat16

B, S, D = 4, 2048, 1024
Q = 1024          # queries (and own keys) per core
SCALE = 1.0 / 32.0
EPS = 1e-6
NKC = S // 128    # 16 key chunks (global)
NDC = D // 128    # 8 d chunks
RG = [[0, 1], [2, 3], [4, 5], [6, 7]]


def _bcast_ap(ap_1d, parts=128):
    """[N] dram AP -> [parts, N] AP with 0-stride partition dim."""
    return bass.AP(
        tensor=ap_1d.tensor, offset=ap_1d.offset, ap=[[0, parts]] + list(ap_1d.ap)
    )


def build_nc():
    nc = bacc.Bacc("TRN2", target_bir_lowering=False, debug=False, num_devices=8)

    xqt = nc.dram_tensor("xqt", [D, Q], BF16, kind="ExternalInput")
    xq = nc.dram_tensor("xq", [Q, D], F32, kind="ExternalInput")
    wq_d = nc.dram_tensor("wq", [D, D], BF16, kind="ExternalInput")
    wk_d = nc.dram_tensor("wk", [D, D], BF16, kind="ExternalInput")
    wv_d = nc.dram_tensor("wv", [D, D], BF16, kind="ExternalInput")
    wo_d = nc.dram_tensor("wo", [D, D], BF16, kind="ExternalInput")
    bqT_d = nc.dram_tensor("bqT", [128, NDC], F32, kind="ExternalInput")
    bkT_d = nc.dram_tensor("bkT", [128, NDC], F32, kind="ExternalInput")
    gamma_d = nc.dram_tensor("gamma", [D], F32, kind="ExternalInput")
    beta_d = nc.dram_tensor("beta", [D], F32, kind="ExternalInput")
    out_d = nc.dram_tensor("out", [Q, D], F32, kind="ExternalOutput")

    with tile.TileContext(nc) as tc:
        with (
            tc.tile_pool(name="small", bufs=1) as p_small,
            tc.tile_pool(name="dram", bufs=1, space="DRAM") as p_dram,
            tc.tile_pool(name="qtsb", bufs=NDC) as p_qt,
            tc.tile_pool(name="ptsb", bufs=NKC) as p_pt,
            tc.tile_pool(name="kto", bufs=NDC) as p_kto,
            tc.tile_pool(name="vow", bufs=NDC) as p_vow,
        ):
            kvin_kt = p_dram.tile([Q, D], BF16, name="kvin_kt")
            kvout_kt = p_dram.tile([S, D], BF16, name="kvout_kt")
            kvin_v = p_dram.tile([Q, D], BF16, name="kvin_v")
            kvout_v = p_dram.tile([S, D], BF16, name="kvout_v")

            # ---- constants / small tiles ----
            bqT = p_small.tile([128, NDC], F32)
            nc.gpsimd.dma_start(out=bqT[:, :], in_=bqT_d[:, :])
            bkT = p_small.tile([128, NDC], F32)
            nc.gpsimd.dma_start(out=bkT[:, :], in_=bkT_d[:, :])
            ones = p_small.tile([128, 1], BF16)
            nc.vector.memset(ones[:, :], 1.0)
            eps_t = p_small.tile([128, 1], F32)
            nc.vector.memset(eps_t[:, :], EPS)
            recip = p_small.tile([128, 8], F32)

            pid = nc.sync.partition_id()
            partner_off = (1 - (pid % 2)) * Q   # partner's row base in gathered buffers

            qt = [p_qt.tile([128, Q], BF16, tag="qt", name=f"qt{i}") for i in range(NDC)]
            pt = [p_pt.tile([128, Q], BF16, tag="pt", name=f"pt{i}") for i in range(NKC)]

            # ---- projections for own half: QT (sbuf), KT/V (to DRAM bounce) ----
            with (
                tc.tile_pool(name="wp", bufs=3 * NDC) as p_w,
                tc.tile_pool(name="xqtp", bufs=NDC) as p_xqt,
                            ):
                wq = [p_w.tile([128, D], BF16, tag="w", name=f"wq{i}") for i in range(NDC)]
                wk = [p_w.tile([128, D], BF16, tag="w", name=f"wk{i}") for i in range(NDC)]
                wv = [p_w.tile([128, D], BF16, tag="w", name=f"wv{i}") for i in range(NDC)]
                xqts = [p_xqt.tile([128, Q], BF16, tag="xqt", name=f"xqts{i}") for i in range(NDC)]
                for dc in range(NDC):
                    nc.sync.dma_start(out=xqts[dc][:, :], in_=xqt[128 * dc:128 * (dc + 1), :])
                    nc.gpsimd.dma_start(out=wk[dc][:, :], in_=wk_d[128 * dc:128 * (dc + 1), :])
                for dc in range(NDC):
                    nc.sync.dma_start(out=wq[dc][:, :], in_=wq_d[128 * dc:128 * (dc + 1), :])
                    nc.gpsimd.dma_start(out=wv[dc][:, :], in_=wv_d[128 * dc:128 * (dc + 1), :])

                # Projections run di (contraction) outermost over 8 PSUM banks so
                # the PE starts as soon as the first (wk, xqt) chunk pair lands.
                with tc.tile_pool(name="psp", bufs=8, space="PSUM") as p_psp:
                    # KT_own[d, k_own] (+bk) -> kvin_kt rows 128*do
                    kt_ts = [p_kto.tile([128, Q], BF16, tag="kto", name=f"ktt{do}") for do in range(NDC)]
                    for kh in range(2):
                        pss = [p_psp.tile([128, 512], F32, tag="psp", name=f"pskt{kh}_{do}") for do in range(NDC)]
                        for di in range(NDC):
                            for do in range(NDC):
                                nc.tensor.matmul(
                                    pss[do][:, :],
                                    wk[di][:, 128 * do:128 * (do + 1)],
                                    xqts[di][:, 512 * kh:512 * (kh + 1)],
                                    start=(di == 0), stop=(di == NDC - 1),
                                )
                        for do in range(NDC):
                            nc.vector.tensor_scalar(
                                out=kt_ts[do][:, 512 * kh:512 * (kh + 1)], in0=pss[do][:, :],
                                scalar1=bkT[:, do:do + 1], scalar2=None,
                                op0=mybir.AluOpType.add,
                            )
                    for do in range(NDC):
                        nc.sync.dma_start(out=kvin_kt[128 * do:128 * (do + 1), :], in_=kt_ts[do][:, :])
                    nc.gpsimd.collective_compute(
                        "AllGather", mybir.AluOpType.bypass, replica_groups=RG,
                        ins=[kvin_kt[:, :].opt()], outs=[kvout_kt[:, :].opt()],
                    )

                    # QT[d,q]
                    for qh in range(2):
                        pss = [p_psp.tile([128, 512], F32, tag="psp", name=f"psqt{qh}_{do}") for do in range(NDC)]
                        for di in range(NDC):
                            for do in range(NDC):
                                nc.tensor.matmul(
                                    pss[do][:, :],
                                    wq[di][:, 128 * do:128 * (do + 1)],
                                    xqts[di][:, 512 * qh:512 * (qh + 1)],
                                    start=(di == 0), stop=(di == NDC - 1),
                                )
                        for do in range(NDC):
                            nc.vector.tensor_scalar(
                                out=qt[do][:, 512 * qh:512 * (qh + 1)], in0=pss[do][:, :],
                                scalar1=bqT[:, do:do + 1], scalar2=None,
                                op0=mybir.AluOpType.add,
                            )

                    # V_own[k_own, d] -> kvin_v rows 128*kc
                    v_ts = [p_vow.tile([128, D], BF16, tag="vow", name=f"vt{kl}") for kl in range(NDC)]
                    for dh in range(2):
                        pss = [p_psp.tile([128, 512], F32, tag="psp", name=f"psv{dh}_{kl}") for kl in range(NDC)]
                        for di in range(NDC):
                            for kl in range(NDC):
                                nc.tensor.matmul(
                                    pss[kl][:, :],
                                    xqts[di][:, 128 * kl:128 * (kl + 1)],
                                    wv[di][:, 512 * dh:512 * (dh + 1)],
                                    start=(di == 0), stop=(di == NDC - 1),
                                )
                        for kl in range(NDC):
                            nc.vector.tensor_copy(v_ts[kl][:, 512 * dh:512 * (dh + 1)], pss[kl][:, :])
                    for kl in range(NDC):
                        nc.gpsimd.dma_start(out=kvin_v[128 * kl:128 * (kl + 1), :], in_=v_ts[kl][:, :])
                    nc.gpsimd.collective_compute(
                        "AllGather", mybir.AluOpType.bypass, replica_groups=RG,
                        ins=[kvin_v[:, :].opt()], outs=[kvout_v[:, :].opt()],
                    )

            # ---- load gathered KT (d-major per half); ST -> exp -> PT ----
            with (
                tc.tile_pool(name="ps", bufs=6, space="PSUM") as p_ps,
                tc.tile_pool(name="ps1", bufs=2, space="PSUM") as p_ps1,
                tc.tile_pool(name="ktl", bufs=NDC) as p_ktl,
                tc.tile_pool(name="vtl", bufs=NDC) as p_vtl,
                tc.tile_pool(name="atp", bufs=NDC) as p_at,
            ):
                # partner-half KT via runtime-parity offset into the gathered buffer
                ktl = [None] * NDC
                for dc in range(NDC):
                    t = p_ktl.tile([128, Q], BF16, tag="ktl", name=f"ktl{dc}")
                    ktl[dc] = t
                    nc.sync.dma_start(
                        out=t[:, :],
                        in_=kvout_kt[bass.ds(partner_off + 128 * dc, 128), :],
                    )
                # ST -> exp -> PT; PT in LOCAL key order: kc 0..7 = own half
                # (from SBUF, no collective dependency), kc 8..15 = partner.
                # Softmax/attention sums over k are order-invariant as long as
                # the V tiles use the same local order.
                for kc in range(NKC):
                    own, kcl = kc < NDC, kc % NDC
                    for qh in range(2):
                        ps = p_ps.tile([128, 512], F32, tag="ps")
                        for dc in range(NDC):
                            lhs = kt_ts[dc] if own else ktl[dc]
                            nc.tensor.matmul(
                                ps[:, :],
                                lhs[:, 128 * kcl:128 * (kcl + 1)],
                                qt[dc][:, 512 * qh:512 * (qh + 1)],
                                start=(dc == 0), stop=(dc == NDC - 1),
                            )
                        nc.scalar.activation(
                            out=pt[kc][:, 512 * qh:512 * (qh + 1)], in_=ps[:, :],
                            func=mybir.ActivationFunctionType.Exp,
                            bias=0.0, scale=SCALE,
                        )

                # ---- AT[d,q] = V^T @ PT ; denom ; O ; LayerNorm ----
                vtl = [p_vtl.tile([128, D], BF16, tag="vtl", name=f"vtl{i}") for i in range(NDC)]
                for kl in range(NDC):
                    nc.sync.dma_start(
                        out=vtl[kl][:, :],
                        in_=kvout_v[bass.ds(partner_off + 128 * kl, 128), :],
                    )
                at = [p_at.tile([128, Q], BF16, tag="at", name=f"at{i}") for i in range(NDC)]
                for qh in range(2):
                    for dc in range(NDC):
                        ps = p_ps.tile([128, 512], F32, tag="ps")
                        for kc in range(NKC):
                            vt = v_ts[kc] if kc < NDC else vtl[kc - NDC]
                            nc.tensor.matmul(
                                ps[:, :],
                                vt[:, 128 * dc:128 * (dc + 1)],
                                pt[kc][:, 512 * qh:512 * (qh + 1)],
                                start=(kc == 0), stop=(kc == NKC - 1),
                            )
                        nc.vector.tensor_copy(at[dc][:, 512 * qh:512 * (qh + 1)], ps[:, :])
                    for qp in range(4 * qh, 4 * qh + 4):
                        ps1 = p_ps1.tile([128, 1], F32, tag="ps1")
                        for kc in range(NKC):
                            nc.tensor.matmul(
                                ps1[:, :],
                                pt[kc][:, 128 * qp:128 * (qp + 1)],
                                ones[:, :],
                                start=(kc == 0), stop=(kc == NKC - 1),
                            )
                        nc.vector.reciprocal(recip[:, qp:qp + 1], ps1[:, :])

                with (
                    tc.tile_pool(name="wop", bufs=NDC) as p_wo,
                    tc.tile_pool(name="xqp", bufs=3) as p_xq,
                    tc.tile_pool(name="vout", bufs=4) as p_vo,
                    tc.tile_pool(name="lnst", bufs=4) as p_ln,
                ):
                    wo = [p_wo.tile([128, D], BF16, tag="wo", name=f"wo{i}") for i in range(NDC)]
                    for dc in range(NDC):
                        nc.gpsimd.dma_start(out=wo[dc][:, :], in_=wo_d[128 * dc:128 * (dc + 1), :])
                    gam = p_small.tile([128, D], F32)
                    nc.gpsimd.dma_start(out=gam[:, :], in_=_bcast_ap(gamma_d[:]))
                    bet = p_small.tile([128, D], F32)
                    nc.gpsimd.dma_start(out=bet[:, :], in_=_bcast_ap(beta_d[:]))

                    for qp in range(8):
                        v = p_vo.tile([128, D], F32, tag="v")
                        sqs = p_vo.tile([128, D], F32, tag="sqs")
                        xqt_ = p_xq.tile([128, D], F32, tag="xq")
                        nc.gpsimd.dma_start(
                            out=xqt_[:, :], in_=xq[128 * qp:128 * (qp + 1), :]
                        )
                        st = p_ln.tile([128, 4], F32, tag="st")
                        for eh in range(2):
                            ps = p_ps.tile([128, 512], F32, tag="ps")
                            for dc in range(NDC):
                                nc.tensor.matmul(
                                    ps[:, :],
                                    at[dc][:, 128 * qp:128 * (qp + 1)],
                                    wo[dc][:, 512 * eh:512 * (eh + 1)],
                                    start=(dc == 0), stop=(dc == NDC - 1),
                                )
                            # v_half = O/denom + xq_aug; accum = row-sum
                            nc.vector.scalar_tensor_tensor(
                                out=v[:, 512 * eh:512 * (eh + 1)], in0=ps[:, :],
                                scalar=recip[:, qp:qp + 1],
                                in1=xqt_[:, 512 * eh:512 * (eh + 1)],
                                op0=mybir.AluOpType.mult, op1=mybir.AluOpType.add,
                                accum_out=st[:, eh:eh + 1],
                            )
                        # E[v^2] via ACT Square + free accum; then mean/var/rstd
                        nc.scalar.activation(
                            out=sqs[:, :], in_=v[:, :],
                            func=mybir.ActivationFunctionType.Square,
                            accum_out=st[:, 2:3],
                        )
                        nc.vector.tensor_add(st[:, 0:1], st[:, 0:1], st[:, 1:2])
                        nc.scalar.mul(st[:, 0:1], st[:, 0:1], 1.0 / D)     # mean
                        nc.scalar.mul(st[:, 2:3], st[:, 2:3], 1.0 / D)     # E[v^2]
                        nc.vector.tensor_mul(st[:, 1:2], st[:, 0:1], st[:, 0:1])
                        nc.vector.tensor_sub(st[:, 2:3], st[:, 2:3], st[:, 1:2])
                        nc.scalar.activation(
                            out=st[:, 2:3], in_=st[:, 2:3],
                            func=mybir.ActivationFunctionType.Sqrt,
                            bias=eps_t[:, :],
                        )
                        nc.vector.reciprocal(st[:, 2:3], st[:, 2:3])       # rstd
                        # out = ((v - mean)*gamma)*rstd + beta  (2 fused DVE ops)
                        nc.vector.scalar_tensor_tensor(
                            out=v[:, :], in0=v[:, :], scalar=st[:, 0:1],
                            in1=gam[:, :],
                            op0=mybir.AluOpType.subtract, op1=mybir.AluOpType.mult,
                        )
                        nc.vector.scalar_tensor_tensor(
                            out=v[:, :], in0=v[:, :], scalar=st[:, 2:3],
                            in1=bet[:, :],
                            op0=mybir.AluOpType.mult, op1=mybir.AluOpType.add,
                        )
                        nc.sync.dma_start(out=out_d[128 * qp:128 * (qp + 1), :], in_=v[:, :])
    nc.compile()
    return nc


_NC_CACHE = None


def make_in_maps(inputs):
    x = np.asarray(inputs["inputs"], np.float32)
    wo = np.asarray(inputs["wo"], np.float32)
    bf = lambda a: np.ascontiguousarray(a).astype(ml_dtypes.bfloat16)
    bo_eff = np.asarray(inputs["bo"], np.float32) + np.asarray(inputs["bv"], np.float32) @ wo
    shared = {
        "wq": bf(inputs["wq"]), "wk": bf(inputs["wk"]),
        "wv": bf(inputs["wv"]), "wo": bf(wo),
        "bqT": np.ascontiguousarray(np.asarray(inputs["bq"], np.float32).reshape(NDC, 128).T),
        "bkT": np.ascontiguousarray(np.asarray(inputs["bk"], np.float32).reshape(NDC, 128).T),
        "gamma": np.asarray(inputs["gamma"], np.float32),
        "beta": np.asarray(inputs["beta"], np.float32),
    }
    in_maps = []
    for c in range(8):
        b, qh = c // 2, c % 2
        xslab = x[b, Q * qh:Q * (qh + 1), :]
        in_maps.append({
            **shared,
            "xqt": bf(xslab.T),
            "xq": np.ascontiguousarray(xslab) + bo_eff[None, :],
        })
    return in_maps


def kernel(**inputs) -> np.ndarray:
    from concourse.bass_utils import run_bass_kernel_spmd

    global _NC_CACHE
    if _NC_CACHE is None:
        _NC_CACHE = build_nc()
    res = run_bass_kernel_spmd(_NC_CACHE, make_in_maps(inputs), core_ids=list(range(8)))
    out = np.empty((B, S, D), np.float32)
    for c in range(8):
        b, qh = c // 2, c % 2
        out[b, Q * qh:Q * (qh + 1), :] = res.results[c]["out"]
    return out



# revision 3
# speedup vs baseline: 1.2357x; 1.2357x over previous
"""Trainium2 Bass kernel for full-embed-dim self-attention + residual LayerNorm.

Problem: B=4, S=2048, D=1024 fp32.
  q/k/v = x@w{q,k,v}+b; scores = q@k^T/sqrt(D); attn = softmax(scores)@v;
  out = LN(x + attn@wo + bo) * gamma + beta.

Sharding: 8 cores = 4 batches x 2 query-halves (1024 queries each). No
collectives: the host ships each core the FULL batch sequence x (both
layouts) plus the associativity-folded weight products
  wqk = wq @ wk^T        (scores = x wqk x^T  + bias terms)
  wvo = wv @ wo          (attn out = (P @ x) @ wvo / denom)
which eliminates the separate K/V projections entirely (6.44e9 vs 8.59e9
MACs per core).

Keys are shipped permuted (core's own query half first) so all tile
offsets are static; softmax sums over keys are order-invariant.

Bias handling (exact): per-query logit offsets cancel in softmax, so only
the bq-via-key term survives: c[k] = x_k @ (wk @ bq), shipped from host
as a per-key exp() bias. bo+bv@wo is folded into the residual on host.

Per-core dataflow (d-on-partitions "transposed" layout):
  TT[do,q] = wqk^T-chunks @ xq^T            1.07e9 MAC
  ST[k,q]  = x^T-chunks^T @ TT  (PSUM)      2.15e9
  PT[k,q]  = exp(ST/32 + c[k])              (logits O(5), no max-sub)
  YT[d,q]  = x-chunks^T @ PT                2.15e9
  denom[q] = ones^T @ PT
  O[q,e]   = YT^T @ wvo  (1.07e9), O*recip/denom + xq_aug, LayerNorm.

Matmul inputs bf16 (1 PE cycle/row; fp32 PSUM accum).
"""

import numpy as np
import ml_dtypes

import concourse.bass as bass
import concourse.mybir as mybir
import concourse.tile as tile
from concourse import bacc

F32 = mybir.dt.float32
BF16 = mybir.dt.bfloat16

B, S, D = 4, 2048, 1024
Q = 1024          # queries per core
SCALE = 1.0 / 32.0
EPS = 1e-6
NKC = S // 128    # 16 key chunks
NDC = D // 128    # 8 d chunks


def _bcast_ap(ap_1d, parts=128):
    """[N] dram AP -> [parts, N] AP with 0-stride partition dim."""
    return bass.AP(
        tensor=ap_1d.tensor, offset=ap_1d.offset, ap=[[0, parts]] + list(ap_1d.ap)
    )


def build_nc():
    nc = bacc.Bacc("TRN2", target_bir_lowering=False, debug=False, num_devices=8)

    xt_d = nc.dram_tensor("xt", [D, S], BF16, kind="ExternalInput")
    xk_d = nc.dram_tensor("xk", [S, D], BF16, kind="ExternalInput")
    wqk_d = nc.dram_tensor("wqk", [D, D], BF16, kind="ExternalInput")
    wvo_d = nc.dram_tensor("wvo", [D, D], BF16, kind="ExternalInput")
    xq_d = nc.dram_tensor("xq", [Q, D], F32, kind="ExternalInput")
    cb_d = nc.dram_tensor("cb", [128, NKC], F32, kind="ExternalInput")
    gamma_d = nc.dram_tensor("gamma", [D], F32, kind="ExternalInput")
    beta_d = nc.dram_tensor("beta", [D], F32, kind="ExternalInput")
    out_d = nc.dram_tensor("out", [Q, D], F32, kind="ExternalOutput")

    with tile.TileContext(nc) as tc:
        with (
            tc.tile_pool(name="small", bufs=1) as p_small,
            tc.tile_pool(name="ptsb", bufs=NKC) as p_pt,
            tc.tile_pool(name="xkp", bufs=NKC) as p_xk,
            tc.tile_pool(name="ytp", bufs=NDC) as p_yt,
        ):
            # ---- constants / small tiles ----
            cb = p_small.tile([128, NKC], F32)
            nc.gpsimd.dma_start(out=cb[:, :], in_=cb_d[:, :])
            ones = p_small.tile([128, 1], BF16)
            nc.vector.memset(ones[:, :], 1.0)
            eps_t = p_small.tile([128, 1], F32)
            nc.vector.memset(eps_t[:, :], EPS)
            recip = p_small.tile([128, 8], F32)

            pt = [p_pt.tile([128, Q], BF16, tag="pt", name=f"pt{i}") for i in range(NKC)]
            yt = [p_yt.tile([128, Q], BF16, tag="yt", name=f"yt{i}") for i in range(NDC)]
            xk = [p_xk.tile([128, D], BF16, tag="xk", name=f"xk{i}") for i in range(NKC)]

            # ---- stages A+B: TT then ST->exp->PT (xt/wqk/tt scoped) ----
            with (
                tc.tile_pool(name="xtp", bufs=NDC) as p_xt,
                tc.tile_pool(name="ttp", bufs=NDC) as p_tt,
                tc.tile_pool(name="psA", bufs=8, space="PSUM") as p_psA,
            ):
                xt = [p_xt.tile([128, S], BF16, tag="xt", name=f"xt{i}") for i in range(NDC)]
                tt = [p_tt.tile([128, Q], BF16, tag="tt", name=f"tt{i}") for i in range(NDC)]
                with tc.tile_pool(name="wqkp", bufs=NDC) as p_wqk:
                    wqk = [p_wqk.tile([128, D], BF16, tag="wqk", name=f"wqk{i}") for i in range(NDC)]
                    for di in range(NDC):
                        nc.sync.dma_start(out=xt[di][:, :], in_=xt_d[128 * di:128 * (di + 1), :])
                        nc.gpsimd.dma_start(out=wqk[di][:, :], in_=wqk_d[128 * di:128 * (di + 1), :])

                    # TT[do, q] accumulated di-outermost over 8 PSUM banks so the
                    # PE starts once the first (wqk, xt) chunk pair lands.
                    # Own queries are key-columns 0..1023 (host permutation).
                    for qh in range(2):
                        pss = [p_psA.tile([128, 512], F32, tag="psA", name=f"pstt{qh}_{do}") for do in range(NDC)]
                        for di in range(NDC):
                            for do in range(NDC):
                                nc.tensor.matmul(
                                    pss[do][:, :],
                                    wqk[di][:, 128 * do:128 * (do + 1)],
                                    xt[di][:, 512 * qh:512 * (qh + 1)],
                                    start=(di == 0), stop=(di == NDC - 1),
                                )
                        for do in range(NDC):
                            nc.vector.tensor_copy(tt[do][:, 512 * qh:512 * (qh + 1)], pss[do][:, :])

                # prefetch x (key-major) for stage C while B runs
                for kc in range(NKC):
                    nc.sync.dma_start(out=xk[kc][:, :], in_=xk_d[128 * kc:128 * (kc + 1), :])

                # ST -> exp -> PT.  qh-outer so stage C's qh=0 deps finish early.
                for qh in range(2):
                    for kc in range(NKC):
                        ps = p_psA.tile([128, 512], F32, tag="psA")
                        for do in range(NDC):
                            nc.tensor.matmul(
                                ps[:, :],
                                xt[do][:, 128 * kc:128 * (kc + 1)],
                                tt[do][:, 512 * qh:512 * (qh + 1)],
                                start=(do == 0), stop=(do == NDC - 1),
                            )
                        nc.scalar.activation(
                            out=pt[kc][:, 512 * qh:512 * (qh + 1)], in_=ps[:, :],
                            func=mybir.ActivationFunctionType.Exp,
                            bias=cb[:, kc:kc + 1], scale=SCALE,
                        )

            # ---- stage C: YT[d, q] = x^T @ PT ----
            with (
                tc.tile_pool(name="wvop", bufs=NDC) as p_wvo,
                tc.tile_pool(name="xqp", bufs=3) as p_xq,
                tc.tile_pool(name="vout", bufs=4) as p_vo,
                tc.tile_pool(name="lnst", bufs=4) as p_ln,
            ):
                wvo = [p_wvo.tile([128, D], BF16, tag="wvo", name=f"wvo{i}") for i in range(NDC)]
                for dc in range(NDC):
                    nc.gpsimd.dma_start(out=wvo[dc][:, :], in_=wvo_d[128 * dc:128 * (dc + 1), :])
                gam = p_small.tile([128, D], F32)
                nc.gpsimd.dma_start(out=gam[:, :], in_=_bcast_ap(gamma_d[:]))
                bet = p_small.tile([128, D], F32)
                nc.gpsimd.dma_start(out=bet[:, :], in_=_bcast_ap(beta_d[:]))

                with tc.tile_pool(name="psC", bufs=8, space="PSUM") as p_psC:
                    # kc-outermost over 8 dc-banks per qh round
                    for qh in range(2):
                        pss = [p_psC.tile([128, 512], F32, tag="psC", name=f"psyt{qh}_{dc}") for dc in range(NDC)]
                        for kc in range(NKC):
                            for dc in range(NDC):
                                nc.tensor.matmul(
                                    pss[dc][:, :],
                                    xk[kc][:, 128 * dc:128 * (dc + 1)],
                                    pt[kc][:, 512 * qh:512 * (qh + 1)],
                                    start=(kc == 0), stop=(kc == NKC - 1),
                                )
                        for dc in range(NDC):
                            nc.vector.tensor_copy(yt[dc][:, 512 * qh:512 * (qh + 1)], pss[dc][:, :])

                # ---- stages D/E: denom, O + residual + LayerNorm ----
                with (
                    tc.tile_pool(name="ps", bufs=6, space="PSUM") as p_ps,
                    tc.tile_pool(name="ps1", bufs=2, space="PSUM") as p_ps1,
                ):
                    for qp in range(8):
                        ps1 = p_ps1.tile([128, 1], F32, tag="ps1")
                        for kc in range(NKC):
                            nc.tensor.matmul(
                                ps1[:, :],
                                pt[kc][:, 128 * qp:128 * (qp + 1)],
                                ones[:, :],
                                start=(kc == 0), stop=(kc == NKC - 1),
                            )
                        nc.vector.reciprocal(recip[:, qp:qp + 1], ps1[:, :])

                    for qp in range(8):
                        v = p_vo.tile([128, D], F32, tag="v")
                        sqs = p_vo.tile([128, D], F32, tag="sqs")
                        xqt_ = p_xq.tile([128, D], F32, tag="xq")
                        nc.gpsimd.dma_start(
                            out=xqt_[:, :], in_=xq_d[128 * qp:128 * (qp + 1), :]
                        )
                        st = p_ln.tile([128, 4], F32, tag="st")
                        for eh in range(2):
                            ps = p_ps.tile([128, 512], F32, tag="ps")
                            for dc in range(NDC):
                                nc.tensor.matmul(
                                    ps[:, :],
                                    yt[dc][:, 128 * qp:128 * (qp + 1)],
                                    wvo[dc][:, 512 * eh:512 * (eh + 1)],
                                    start=(dc == 0), stop=(dc == NDC - 1),
                                )
                            # v_half = O/denom + xq_aug; accum = row-sum
                            nc.vector.scalar_tensor_tensor(
                                out=v[:, 512 * eh:512 * (eh + 1)], in0=ps[:, :],
                                scalar=recip[:, qp:qp + 1],
                                in1=xqt_[:, 512 * eh:512 * (eh + 1)],
                                op0=mybir.AluOpType.mult, op1=mybir.AluOpType.add,
                                accum_out=st[:, eh:eh + 1],
                            )
                        # E[v^2] via ACT Square + free accum; then mean/var/rstd
                        nc.scalar.activation(
                            out=sqs[:, :], in_=v[:, :],
                            func=mybir.ActivationFunctionType.Square,
                            accum_out=st[:, 2:3],
                        )
                        nc.vector.tensor_add(st[:, 0:1], st[:, 0:1], st[:, 1:2])
                        nc.scalar.mul(st[:, 0:1], st[:, 0:1], 1.0 / D)     # mean
                        nc.scalar.mul(st[:, 2:3], st[:, 2:3], 1.0 / D)     # E[v^2]
                        nc.vector.tensor_mul(st[:, 1:2], st[:, 0:1], st[:, 0:1])
                        nc.vector.tensor_sub(st[:, 2:3], st[:, 2:3], st[:, 1:2])
                        nc.scalar.activation(
                            out=st[:, 2:3], in_=st[:, 2:3],
                            func=mybir.ActivationFunctionType.Sqrt,
                            bias=eps_t[:, :],
                        )
                        nc.vector.reciprocal(st[:, 2:3], st[:, 2:3])       # rstd
                        # out = ((v - mean)*gamma)*rstd + beta  (2 fused DVE ops)
                        nc.vector.scalar_tensor_tensor(
                            out=v[:, :], in0=v[:, :], scalar=st[:, 0:1],
                            in1=gam[:, :],
                            op0=mybir.AluOpType.subtract, op1=mybir.AluOpType.mult,
                        )
                        nc.vector.scalar_tensor_tensor(
                            out=v[:, :], in0=v[:, :], scalar=st[:, 2:3],
                            in1=bet[:, :],
                            op0=mybir.AluOpType.mult, op1=mybir.AluOpType.add,
                        )
                        nc.sync.dma_start(out=out_d[128 * qp:128 * (qp + 1), :], in_=v[:, :])
    nc.compile()
    return nc


_NC_CACHE = None


def make_in_maps(inputs):
    x = np.asarray(inputs["inputs"], np.float32)
    wq = np.asarray(inputs["wq"], np.float32)
    wk = np.asarray(inputs["wk"], np.float32)
    wv = np.asarray(inputs["wv"], np.float32)
    wo = np.asarray(inputs["wo"], np.float32)
    bq = np.asarray(inputs["bq"], np.float32)
    bf = lambda a: np.ascontiguousarray(a).astype(ml_dtypes.bfloat16)
    bo_eff = np.asarray(inputs["bo"], np.float32) + np.asarray(inputs["bv"], np.float32) @ wo
    wqk = wq @ wk.T
    wvo = wv @ wo
    kcv = wk @ bq  # per-key logit offset direction: c[k] = x_k . kcv
    shared = {
        "wqk": bf(wqk), "wvo": bf(wvo),
        "gamma": np.asarray(inputs["gamma"], np.float32),
        "beta": np.asarray(inputs["beta"], np.float32),
    }
    in_maps = []
    for c in range(8):
        b, qh = c // 2, c % 2
        xb = x[b]                                  # [S, D]
        own = xb[Q * qh:Q * (qh + 1), :]
        oth = xb[Q * (1 - qh):Q * (2 - qh), :]
        xbp = np.concatenate([own, oth], axis=0)   # keys permuted: own half first
        cb = (SCALE * (xbp @ kcv)).reshape(NKC, 128).T
        in_maps.append({
            **shared,
            "xt": bf(xbp.T),
            "xk": bf(xbp),
            "xq": np.ascontiguousarray(own) + bo_eff[None, :],
            "cb": np.ascontiguousarray(cb),
        })
    return in_maps


def kernel(**inputs) -> np.ndarray:
    from concourse.bass_utils import run_bass_kernel_spmd

    global _NC_CACHE
    if _NC_CACHE is None:
        _NC_CACHE = build_nc()
    res = run_bass_kernel_spmd(_NC_CACHE, make_in_maps(inputs), core_ids=list(range(8)))
    out = np.empty((B, S, D), np.float32)
    for c in range(8):
        b, qh = c // 2, c % 2
        out[b, Q * qh:Q * (qh + 1), :] = res.results[c]["out"]
    return out


# revision 12
# speedup vs baseline: 1.2932x; 1.0465x over previous
"""Trainium2 Bass kernel for full-embed-dim self-attention + residual LayerNorm.

Problem: B=4, S=2048, D=1024 fp32.
  q/k/v = x@w{q,k,v}+b; scores = q@k^T/sqrt(D); attn = softmax(scores)@v;
  out = LN(x + attn@wo + bo) * gamma + beta.

Sharding: 8 cores = 4 batches x 2 query-halves (1024 queries each). No
collectives: the host ships each core the FULL batch sequence x (both
layouts) plus the associativity-folded weight products
  wqk = wq @ wk^T        (scores = x wqk x^T  + bias terms)
  wvo = wv @ wo          (attn out = (P @ x) @ wvo / denom)
which eliminates the separate K/V projections entirely (6.44e9 vs 8.59e9
MACs per core).

Keys are shipped permuted (core's own query half first) so all tile
offsets are static; softmax sums over keys are order-invariant.

Bias handling (exact): per-query logit offsets cancel in softmax, so only
the bq-via-key term survives: c[k] = x_k @ (wk @ bq), shipped from host
as a per-key exp() bias. bo+bv@wo is folded into the residual on host.

Per-core dataflow (d-on-partitions "transposed" layout):
  TT[do,q] = wqk^T-chunks @ xq^T            1.07e9 MAC
  ST[k,q]  = x^T-chunks^T @ TT  (PSUM)      2.15e9
  PT[k,q]  = exp(ST/32 + c[k])              (logits O(5), no max-sub)
  YT[d,q]  = x-chunks^T @ PT                2.15e9
  denom[q] = ones^T @ PT
  O[q,e]   = YT^T @ wvo  (1.07e9), O*recip/denom + xq_aug, LayerNorm.

Matmul inputs bf16 (1 PE cycle/row; fp32 PSUM accum).
"""

import numpy as np
import ml_dtypes

import concourse.bass as bass
import concourse.mybir as mybir
import concourse.tile as tile
from concourse import bacc

F32 = mybir.dt.float32
BF16 = mybir.dt.bfloat16

B, S, D = 4, 2048, 1024
Q = 1024          # queries per core
SCALE = 1.0 / 32.0
EPS = 1e-6
NKC = S // 128    # 16 key chunks
NDC = D // 128    # 8 d chunks


def _bcast_ap(ap_1d, parts=128):
    """[N] dram AP -> [parts, N] AP with 0-stride partition dim."""
    return bass.AP(
        tensor=ap_1d.tensor, offset=ap_1d.offset, ap=[[0, parts]] + list(ap_1d.ap)
    )


def build_nc():
    nc = bacc.Bacc("TRN2", target_bir_lowering=False, debug=False, num_devices=8)

    xt_d = nc.dram_tensor("xt", [D, S], BF16, kind="ExternalInput")
    xk_d = nc.dram_tensor("xk", [S, D], BF16, kind="ExternalInput")
    wqk_d = nc.dram_tensor("wqk", [D, D], BF16, kind="ExternalInput")
    wvo_d = nc.dram_tensor("wvo", [D, D], BF16, kind="ExternalInput")
    xq_d = nc.dram_tensor("xq", [Q, D], F32, kind="ExternalInput")
    cb_d = nc.dram_tensor("cb", [128, NKC], F32, kind="ExternalInput")
    gamma_d = nc.dram_tensor("gamma", [D], BF16, kind="ExternalInput")
    beta_d = nc.dram_tensor("beta", [D], BF16, kind="ExternalInput")
    out_d = nc.dram_tensor("out", [Q, D], BF16, kind="ExternalOutput")

    with tile.TileContext(nc) as tc:
        with (
            tc.tile_pool(name="small", bufs=1) as p_small,
            tc.tile_pool(name="ptsb", bufs=NKC) as p_pt,
            tc.tile_pool(name="xkp", bufs=NKC) as p_xk,
            tc.tile_pool(name="ytp", bufs=NDC) as p_yt,
        ):
            # ---- constants / small tiles ----
            cb = p_small.tile([128, NKC], F32)
            nc.gpsimd.dma_start(out=cb[:, :], in_=cb_d[:, :])
            ones = p_small.tile([128, 1], BF16)
            nc.vector.memset(ones[:, :], 1.0)
            eps_t = p_small.tile([128, 1], F32)
            nc.vector.memset(eps_t[:, :], EPS)
            recip = p_small.tile([128, 8], F32)

            pt = [p_pt.tile([128, Q], BF16, tag="pt", name=f"pt{i}") for i in range(NKC)]
            yt = [p_yt.tile([128, Q], BF16, tag="yt", name=f"yt{i}") for i in range(NDC)]
            xk = [p_xk.tile([128, D], BF16, tag="xk", name=f"xk{i}") for i in range(NKC)]

            # ---- stages A+B: TT then ST->exp->PT (xt/wqk/tt scoped) ----
            with (
                tc.tile_pool(name="xtp", bufs=NDC) as p_xt,
                tc.tile_pool(name="ttp", bufs=NDC) as p_tt,
                tc.tile_pool(name="psA", bufs=8, space="PSUM") as p_psA,
            ):
                xt = [p_xt.tile([128, S], BF16, tag="xt", name=f"xt{i}") for i in range(NDC)]
                tt = [p_tt.tile([128, Q], BF16, tag="tt", name=f"tt{i}") for i in range(NDC)]
                with tc.tile_pool(name="wqkp", bufs=NDC) as p_wqk:
                    wqk = [p_wqk.tile([128, D], BF16, tag="wqk", name=f"wqk{i}") for i in range(NDC)]
                    # Stage A only needs xt cols 0..1023 (own queries, permuted
                    # first): ship those halves first so the PE starts sooner.
                    for di in range(NDC):
                        nc.sync.dma_start(out=xt[di][:, 0:Q], in_=xt_d[128 * di:128 * (di + 1), 0:Q])
                        nc.gpsimd.dma_start(out=wqk[di][:, :], in_=wqk_d[128 * di:128 * (di + 1), :])
                    for di in range(NDC):
                        nc.sync.dma_start(out=xt[di][:, Q:S], in_=xt_d[128 * di:128 * (di + 1), Q:S])

                    # TT[do, q] accumulated di-outermost over 8 PSUM banks so the
                    # PE starts once the first (wqk, xt) chunk pair lands.
                    # Own queries are key-columns 0..1023 (host permutation).
                    for qh in range(2):
                        pss = [p_psA.tile([128, 512], F32, tag="psA", name=f"pstt{qh}_{do}") for do in range(NDC)]
                        for di in range(NDC):
                            for do in range(NDC):
                                nc.tensor.matmul(
                                    pss[do][:, :],
                                    wqk[di][:, 128 * do:128 * (do + 1)],
                                    xt[di][:, 512 * qh:512 * (qh + 1)],
                                    start=(di == 0), stop=(di == NDC - 1),
                                )
                        for do in range(NDC):
                            nc.vector.tensor_copy(tt[do][:, 512 * qh:512 * (qh + 1)], pss[do][:, :])

                # prefetch x (key-major) for stage C while B runs
                for kc in range(NKC):
                    nc.sync.dma_start(out=xk[kc][:, :], in_=xk_d[128 * kc:128 * (kc + 1), :])

                # ST -> exp -> PT.  qh-outer so stage C's qh=0 deps finish early.
                for qh in range(2):
                    for kc in range(NKC):
                        ps = p_psA.tile([128, 512], F32, tag="psA")
                        for do in range(NDC):
                            nc.tensor.matmul(
                                ps[:, :],
                                xt[do][:, 128 * kc:128 * (kc + 1)],
                                tt[do][:, 512 * qh:512 * (qh + 1)],
                                start=(do == 0), stop=(do == NDC - 1),
                            )
                        nc.scalar.activation(
                            out=pt[kc][:, 512 * qh:512 * (qh + 1)], in_=ps[:, :],
                            func=mybir.ActivationFunctionType.Exp,
                            bias=cb[:, kc:kc + 1], scale=SCALE,
                        )

            # ---- stage C: YT[d, q] = x^T @ PT ----
            with (
                tc.tile_pool(name="wvop", bufs=NDC) as p_wvo,
                tc.tile_pool(name="xqp", bufs=3) as p_xq,
                tc.tile_pool(name="vout", bufs=4) as p_vo,
                tc.tile_pool(name="lnst", bufs=4) as p_ln,
            ):
                wvo = [p_wvo.tile([128, D], BF16, tag="wvo", name=f"wvo{i}") for i in range(NDC)]
                for dc in range(NDC):
                    nc.gpsimd.dma_start(out=wvo[dc][:, :], in_=wvo_d[128 * dc:128 * (dc + 1), :])
                gam = p_small.tile([128, D], BF16)
                nc.gpsimd.dma_start(out=gam[:, :], in_=_bcast_ap(gamma_d[:]))
                bet = p_small.tile([128, D], BF16)
                nc.gpsimd.dma_start(out=bet[:, :], in_=_bcast_ap(beta_d[:]))

                with tc.tile_pool(name="psC", bufs=8, space="PSUM") as p_psC:
                    # kc-inner, one psum at a time: bank reuse distance 8 so the
                    # DVE cast always finishes before the slot cycles back.
                    for qh in range(2):
                        for dc in range(NDC):
                            ps = p_psC.tile([128, 512], F32, tag="psC")
                            for kc in range(NKC):
                                nc.tensor.matmul(
                                    ps[:, :],
                                    xk[kc][:, 128 * dc:128 * (dc + 1)],
                                    pt[kc][:, 512 * qh:512 * (qh + 1)],
                                    start=(kc == 0), stop=(kc == NKC - 1),
                                )
                            nc.vector.tensor_copy(yt[dc][:, 512 * qh:512 * (qh + 1)], ps[:, :])

                # ---- stages D/E: denom, O + residual + LayerNorm ----
                with (
                    tc.tile_pool(name="ps", bufs=6, space="PSUM") as p_ps,
                    tc.tile_pool(name="ps1", bufs=2, space="PSUM") as p_ps1,
                ):
                    for qp in range(8):
                        ps1 = p_ps1.tile([128, 1], F32, tag="ps1")
                        for kc in range(NKC):
                            nc.tensor.matmul(
                                ps1[:, :],
                                pt[kc][:, 128 * qp:128 * (qp + 1)],
                                ones[:, :],
                                start=(kc == 0), stop=(kc == NKC - 1),
                            )
                        nc.vector.reciprocal(recip[:, qp:qp + 1], ps1[:, :])

                    for qp in range(8):
                        v = p_vo.tile([128, D], BF16, tag="v")
                        sqs = p_vo.tile([128, D], BF16, tag="sqs")
                        xqt_ = p_xq.tile([128, D], F32, tag="xq")
                        nc.gpsimd.dma_start(
                            out=xqt_[:, :], in_=xq_d[128 * qp:128 * (qp + 1), :]
                        )
                        st = p_ln.tile([128, 4], F32, tag="st")
                        for eh in range(2):
                            ps = p_ps.tile([128, 512], F32, tag="ps")
                            for dc in range(NDC):
                                nc.tensor.matmul(
                                    ps[:, :],
                                    yt[dc][:, 128 * qp:128 * (qp + 1)],
                                    wvo[dc][:, 512 * eh:512 * (eh + 1)],
                                    start=(dc == 0), stop=(dc == NDC - 1),
                                )
                            # v_half = O/denom + xq_aug; accum = row-sum
                            nc.vector.scalar_tensor_tensor(
                                out=v[:, 512 * eh:512 * (eh + 1)], in0=ps[:, :],
                                scalar=recip[:, qp:qp + 1],
                                in1=xqt_[:, 512 * eh:512 * (eh + 1)],
                                op0=mybir.AluOpType.mult, op1=mybir.AluOpType.add,
                                accum_out=st[:, eh:eh + 1],
                            )
                        # E[v^2] via ACT Square + free accum; then mean/var/rstd
                        nc.scalar.activation(
                            out=sqs[:, :], in_=v[:, :],
                            func=mybir.ActivationFunctionType.Square,
                            accum_out=st[:, 2:3],
                        )
                        nc.vector.tensor_add(st[:, 0:1], st[:, 0:1], st[:, 1:2])
                        nc.scalar.mul(st[:, 0:1], st[:, 0:1], 1.0 / D)     # mean
                        nc.scalar.mul(st[:, 2:3], st[:, 2:3], 1.0 / D)     # E[v^2]
                        nc.vector.tensor_mul(st[:, 1:2], st[:, 0:1], st[:, 0:1])
                        nc.vector.tensor_sub(st[:, 2:3], st[:, 2:3], st[:, 1:2])
                        nc.scalar.activation(
                            out=st[:, 2:3], in_=st[:, 2:3],
                            func=mybir.ActivationFunctionType.Sqrt,
                            bias=eps_t[:, :],
                        )
                        nc.vector.reciprocal(st[:, 2:3], st[:, 2:3])       # rstd
                        # out = ((v - mean)*gamma)*rstd + beta  (2 fused DVE ops)
                        nc.vector.scalar_tensor_tensor(
                            out=sqs[:, :], in0=v[:, :], scalar=st[:, 0:1],
                            in1=gam[:, :],
                            op0=mybir.AluOpType.subtract, op1=mybir.AluOpType.mult,
                        )
                        nc.vector.scalar_tensor_tensor(
                            out=v[:, :], in0=sqs[:, :], scalar=st[:, 2:3],
                            in1=bet[:, :],
                            op0=mybir.AluOpType.mult, op1=mybir.AluOpType.add,
                        )
                        nc.sync.dma_start(out=out_d[128 * qp:128 * (qp + 1), :], in_=v[:, :])
    nc.compile()
    return nc


_NC_CACHE = None


def make_in_maps(inputs):
    x = np.asarray(inputs["inputs"], np.float32)
    wq = np.asarray(inputs["wq"], np.float32)
    wk = np.asarray(inputs["wk"], np.float32)
    wv = np.asarray(inputs["wv"], np.float32)
    wo = np.asarray(inputs["wo"], np.float32)
    bq = np.asarray(inputs["bq"], np.float32)
    bf = lambda a: np.ascontiguousarray(a).astype(ml_dtypes.bfloat16)
    bo_eff = np.asarray(inputs["bo"], np.float32) + np.asarray(inputs["bv"], np.float32) @ wo
    wqk = wq @ wk.T
    wvo = wv @ wo
    kcv = wk @ bq  # per-key logit offset direction: c[k] = x_k . kcv
    shared = {
        "wqk": bf(wqk), "wvo": bf(wvo),
        "gamma": bf(inputs["gamma"]),
        "beta": bf(inputs["beta"]),
    }
    in_maps = []
    for c in range(8):
        b, qh = c // 2, c % 2
        xb = x[b]                                  # [S, D]
        own = xb[Q * qh:Q * (qh + 1), :]
        oth = xb[Q * (1 - qh):Q * (2 - qh), :]
        xbp = np.concatenate([own, oth], axis=0)   # keys permuted: own half first
        cb = (SCALE * (xbp @ kcv)).reshape(NKC, 128).T
        in_maps.append({
            **shared,
            "xt": bf(xbp.T),
            "xk": bf(xbp),
            "xq": np.ascontiguousarray(own) + bo_eff[None, :],
            "cb": np.ascontiguousarray(cb),
        })
    return in_maps


def kernel(**inputs) -> np.ndarray:
    from concourse.bass_utils import run_bass_kernel_spmd

    global _NC_CACHE
    if _NC_CACHE is None:
        _NC_CACHE = build_nc()
    res = run_bass_kernel_spmd(_NC_CACHE, make_in_maps(inputs), core_ids=list(range(8)))
    out = np.empty((B, S, D), np.float32)
    for c in range(8):
        b, qh = c // 2, c % 2
        out[b, Q * qh:Q * (qh + 1), :] = res.results[c]["out"].astype(np.float32)
    return out


# revision 15
# speedup vs baseline: 2.2111x; 1.7097x over previous
"""Trainium2 Bass kernel: full-embed-dim self-attention + residual LayerNorm.

Problem: B=4, S=2048, D=1024 fp32.
  q/k/v = x@w{q,k,v}+b; scores = q@k^T/sqrt(D); attn = softmax(scores)@v;
  out = LN(x + attn@wo + bo) * gamma + beta.

Sharding: 8 cores = 4 batches x 2 query-halves (1024 queries each), no
collectives: each core receives the full batch sequence x plus the
associativity-folded weight products
  wqk = wq @ wk^T   (scores = x wqk x^T)
  wvo = wv @ wo     (attn out = (P @ x) @ wvo / denom)
eliminating the K/V projections (6.44e9 vs 8.59e9 MACs per core). Keys
are shipped permuted (own query half first) so offsets are static;
softmax reductions are key-order invariant.

All four big matmul groups run in fp8(e4m3) DoubleRow mode: operands are
pair-packed [128, 2, N] so each PE instruction contracts 256 rows
(~1.44x bf16 throughput at FD=512). Host scales (8x wqk, 16x wvo) keep
fp8 operands in the normal range; exp() absorbs 1/(32*8) and a -3 bias
so PT fits e4m3's 240 max; the 16x on wvo cancels via ones=16 in the
denominator matmul. fp32 PSUM throughout; residual + LayerNorm in fp32
with only the final store rounded to bf16 (host upcasts).

Per-core dataflow (d-on-partitions):
  TT[do,q]  = wqk^T @ xq^T                      (A: 64 DR-matmuls)
  ST[k,q]   = x^T^T @ TT ; PT = exp(ST/256+cb)  (B: 128)
  den[q]    = 16*ones^T @ PT  -> DRAM-transposed -> recip[q-part]
  YT[d,q]   = x^T @ PT                          (C: 128)
  O[q,e]    = YT^T @ wvo                        (E: 64), then
  v = O*recip + xq_aug; LayerNorm (fused tensor_scalar when gamma==1,
  beta==0, else general two-op path).
"""

import numpy as np
import ml_dtypes

import concourse.bass as bass
import concourse.mybir as mybir
import concourse.tile as tile
from concourse import bacc

F32 = mybir.dt.float32
BF16 = mybir.dt.bfloat16
FP8 = mybir.dt.float8e4
DR = mybir.MatmulPerfMode.DoubleRow
NPF8 = ml_dtypes.float8_e4m3

B, S, D = 4, 2048, 1024
Q = 1024            # queries per core
WQK_S = 8.0         # host scale on wqk (fp8 range)
WVO_S = 16.0        # host scale on wvo; cancelled by ones=16 in denom
SCALE = 1.0 / (32.0 * WQK_S)   # exp scale: 1/sqrt(D) / WQK_S
CSHIFT = 3.0        # logit shift so PT max ~e^2.5 << 240 (e4m3 max)
EPS = 1e-6
NKC = S // 128      # 16 key chunks
NDC = D // 128      # 8 d chunks
NKP = NKC // 2      # 8 key pairs
NDP = NDC // 2      # 4 d pairs


def _bcast_ap(ap_1d, parts=128):
    """[N] dram AP -> [parts, N] AP with 0-stride partition dim."""
    return bass.AP(
        tensor=ap_1d.tensor, offset=ap_1d.offset, ap=[[0, parts]] + list(ap_1d.ap)
    )


def _transpose_ap(ap_1d, parts=128, cols=8):
    """[parts*cols] dram AP -> [parts, cols] AP: out[p, c] = in[c*parts + p]."""
    return bass.AP(
        tensor=ap_1d.tensor, offset=ap_1d.offset, ap=[[1, parts], [parts, cols]]
    )


def build_nc(fused_ln: bool):
    nc = bacc.Bacc("TRN2", target_bir_lowering=False, debug=False, num_devices=8)

    xtp_d = nc.dram_tensor("xtp", [4 * 128, 2, S], FP8, kind="ExternalInput")
    xkp_d = nc.dram_tensor("xkp", [8 * 128, 2, D], FP8, kind="ExternalInput")
    wqkp_d = nc.dram_tensor("wqkp", [4 * 128, 2, D], FP8, kind="ExternalInput")
    wvop_d = nc.dram_tensor("wvop", [4 * 128, 2, D], FP8, kind="ExternalInput")
    xq_d = nc.dram_tensor("xq", [Q, D], F32, kind="ExternalInput")
    cb_d = nc.dram_tensor("cb", [128, NKC], F32, kind="ExternalInput")
    gamma_d = nc.dram_tensor("gamma", [D], BF16, kind="ExternalInput")
    beta_d = nc.dram_tensor("beta", [D], BF16, kind="ExternalInput")
    out_d = nc.dram_tensor("out", [Q, D], BF16, kind="ExternalOutput")

    with tile.TileContext(nc) as tc:
        with (
            tc.tile_pool(name="small", bufs=1) as p_small,
            tc.tile_pool(name="dram", bufs=1, space="DRAM") as p_dram,
            tc.tile_pool(name="ptp", bufs=NKP) as p_pt,
            tc.tile_pool(name="xkp", bufs=NKP) as p_xk,
            tc.tile_pool(name="ytp", bufs=NDP) as p_yt,
        ):
            den_dram = p_dram.tile([Q], F32, name="den_dram")

            cb = p_small.tile([128, NKC], F32)
            nc.gpsimd.dma_start(out=cb[:, :], in_=cb_d[:, :])
            ones16 = p_small.tile([128, 2, 128], FP8)
            nc.vector.memset(ones16[:, :, :], WVO_S)
            eps_t = p_small.tile([128, 1], F32)
            nc.vector.memset(eps_t[:, :], EPS)
            den_sb = p_small.tile([1, Q], F32)
            recip = p_small.tile([128, 8], F32)

            ptp = [p_pt.tile([128, 2, Q], FP8, tag="ptp", name=f"ptp{i}") for i in range(NKP)]
            ytp = [p_yt.tile([128, 2, Q], FP8, tag="ytp", name=f"ytp{i}") for i in range(NDP)]
            xkp = [p_xk.tile([128, 2, D], FP8, tag="xkp", name=f"xkp{i}") for i in range(NKP)]

            # ---- stages A+B(+D): TT, then ST->exp->PT, then denominators ----
            with (
                tc.tile_pool(name="xtp", bufs=NDP) as p_xt,
                tc.tile_pool(name="ttp", bufs=NDP) as p_tt,
                tc.tile_pool(name="psA", bufs=8, space="PSUM") as p_psA,
            ):
                xtp = [p_xt.tile([128, 2, S], FP8, tag="xtp", name=f"xtp{i}") for i in range(NDP)]
                ttp = [p_tt.tile([128, 2, Q], FP8, tag="ttp", name=f"ttp{i}") for i in range(NDP)]
                with tc.tile_pool(name="wqkp", bufs=NDP) as p_wqk:
                    wqkp = [p_wqk.tile([128, 2, D], FP8, tag="wqkp", name=f"wqkp{i}") for i in range(NDP)]
                    # Stage A uses only xt cols 0..1023 (own queries first in
                    # the key permutation): ship those halves first.
                    for j in range(NDP):
                        nc.sync.dma_start(out=xtp[j][:, :, 0:Q], in_=xtp_d[128 * j:128 * (j + 1), :, 0:Q])
                        nc.gpsimd.dma_start(out=wqkp[j][:, :, :], in_=wqkp_d[128 * j:128 * (j + 1), :, :])
                    for j in range(NDP):
                        nc.sync.dma_start(out=xtp[j][:, :, Q:S], in_=xtp_d[128 * j:128 * (j + 1), :, Q:S])

                    # TT: j-outermost over 8 PSUM banks so the PE starts once
                    # the first (wqkp, xtp) pair lands.
                    for qh in range(2):
                        pss = [p_psA.tile([128, 512], F32, tag="psA", name=f"pstt{qh}_{do}") for do in range(NDC)]
                        for j in range(NDP):
                            for do in range(NDC):
                                nc.tensor.matmul(
                                    pss[do][:, :],
                                    wqkp[j][:, :, 128 * do:128 * (do + 1)],
                                    xtp[j][:, :, 512 * qh:512 * (qh + 1)],
                                    start=(j == 0), stop=(j == NDP - 1),
                                    perf_mode=DR,
                                )
                        for do in range(NDC):
                            nc.vector.tensor_copy(ttp[do // 2][:, do % 2, 512 * qh:512 * (qh + 1)], pss[do][:, :])

                # prefetch x (key-major) for stage C while B runs
                for j in range(NKP):
                    nc.sync.dma_start(out=xkp[j][:, :, :], in_=xkp_d[128 * j:128 * (j + 1), :, :])

                # ST -> exp -> PT
                for qh in range(2):
                    for kc in range(NKC):
                        ps = p_psA.tile([128, 512], F32, tag="psA")
                        for j in range(NDP):
                            nc.tensor.matmul(
                                ps[:, :],
                                xtp[j][:, :, 128 * kc:128 * (kc + 1)],
                                ttp[j][:, :, 512 * qh:512 * (qh + 1)],
                                start=(j == 0), stop=(j == NDP - 1),
                                perf_mode=DR,
                            )
                        nc.scalar.activation(
                            out=ptp[kc // 2][:, kc % 2, 512 * qh:512 * (qh + 1)], in_=ps[:, :],
                            func=mybir.ActivationFunctionType.Exp,
                            bias=cb[:, kc:kc + 1], scale=SCALE,
                        )

                # denominators: ones(=16) lhsT duplicates den[q] on all
                # partitions; bounce via DRAM to transpose to [q-part, 8].
                for qh in range(2):
                    psd = p_psA.tile([128, 512], F32, tag="psA")
                    for j in range(NKP):
                        nc.tensor.matmul(
                            psd[:, :],
                            ones16[:, :, :],
                            ptp[j][:, :, 512 * qh:512 * (qh + 1)],
                            start=(j == 0), stop=(j == NKP - 1),
                            perf_mode=DR,
                        )
                    nc.vector.tensor_copy(den_sb[0:1, 512 * qh:512 * (qh + 1)], psd[0:1, :])
                nc.sync.dma_start(out=den_dram[:], in_=den_sb[0:1, :])
                nc.sync.dma_start(out=recip[:, :], in_=_transpose_ap(den_dram[:]))
                nc.vector.reciprocal(recip[:, :], recip[:, :])

            # ---- stage C: YT[d, q] = x^T @ PT ----
            with (
                tc.tile_pool(name="wvop", bufs=NDP) as p_wvo,
                tc.tile_pool(name="xqp", bufs=3) as p_xq,
                tc.tile_pool(name="vout", bufs=6) as p_vo,
                tc.tile_pool(name="lnst", bufs=4) as p_ln,
            ):
                wvop = [p_wvo.tile([128, 2, D], FP8, tag="wvop", name=f"wvop{i}") for i in range(NDP)]
                for j in range(NDP):
                    nc.gpsimd.dma_start(out=wvop[j][:, :, :], in_=wvop_d[128 * j:128 * (j + 1), :, :])
                gam = p_small.tile([128, D], BF16)
                nc.gpsimd.dma_start(out=gam[:, :], in_=_bcast_ap(gamma_d[:]))
                bet = p_small.tile([128, D], BF16)
                nc.gpsimd.dma_start(out=bet[:, :], in_=_bcast_ap(beta_d[:]))

                with tc.tile_pool(name="psC", bufs=8, space="PSUM") as p_psC:
                    for qh in range(2):
                        for dc in range(NDC):
                            ps = p_psC.tile([128, 512], F32, tag="psC")
                            for j in range(NKP):
                                nc.tensor.matmul(
                                    ps[:, :],
                                    xkp[j][:, :, 128 * dc:128 * (dc + 1)],
                                    ptp[j][:, :, 512 * qh:512 * (qh + 1)],
                                    start=(j == 0), stop=(j == NKP - 1),
                                    perf_mode=DR,
                                )
                            nc.vector.tensor_copy(ytp[dc // 2][:, dc % 2, 512 * qh:512 * (qh + 1)], ps[:, :])

                # ---- stage E: O = YT^T @ wvo; v = O*recip + xq; LayerNorm ----
                with tc.tile_pool(name="ps", bufs=8, space="PSUM") as p_ps:
                    for qp in range(8):
                        v = p_vo.tile([128, D], F32, tag="v")
                        sqs = p_vo.tile([128, D], F32, tag="sqs")
                        vout = p_vo.tile([128, D], BF16, tag="vout")
                        xqt_ = p_xq.tile([128, D], F32, tag="xq")
                        nc.gpsimd.dma_start(
                            out=xqt_[:, :], in_=xq_d[128 * qp:128 * (qp + 1), :]
                        )
                        st = p_ln.tile([128, 4], F32, tag="st")
                        for eh in range(2):
                            ps = p_ps.tile([128, 512], F32, tag="ps")
                            for j in range(NDP):
                                nc.tensor.matmul(
                                    ps[:, :],
                                    ytp[j][:, :, 128 * qp:128 * (qp + 1)],
                                    wvop[j][:, :, 512 * eh:512 * (eh + 1)],
                                    start=(j == 0), stop=(j == NDP - 1),
                                    perf_mode=DR,
                                )
                            # v_half = O/(16*denom) + xq_aug; accum = row-sum
                            nc.vector.scalar_tensor_tensor(
                                out=v[:, 512 * eh:512 * (eh + 1)], in0=ps[:, :],
                                scalar=recip[:, qp:qp + 1],
                                in1=xqt_[:, 512 * eh:512 * (eh + 1)],
                                op0=mybir.AluOpType.mult, op1=mybir.AluOpType.add,
                                accum_out=st[:, eh:eh + 1],
                            )
                        # E[v^2] via ACT Square + free accum; then mean/var/rstd
                        nc.scalar.activation(
                            out=sqs[:, :], in_=v[:, :],
                            func=mybir.ActivationFunctionType.Square,
                            accum_out=st[:, 2:3],
                        )
                        nc.vector.tensor_add(st[:, 0:1], st[:, 0:1], st[:, 1:2])
                        nc.scalar.mul(st[:, 0:1], st[:, 0:1], 1.0 / D)     # mean
                        nc.scalar.mul(st[:, 2:3], st[:, 2:3], 1.0 / D)     # E[v^2]
                        nc.vector.tensor_mul(st[:, 1:2], st[:, 0:1], st[:, 0:1])
                        nc.vector.tensor_sub(st[:, 2:3], st[:, 2:3], st[:, 1:2])
                        nc.scalar.activation(
                            out=st[:, 2:3], in_=st[:, 2:3],
                            func=mybir.ActivationFunctionType.Sqrt,
                            bias=eps_t[:, :],
                        )
                        nc.vector.reciprocal(st[:, 2:3], st[:, 2:3])       # rstd
                        if fused_ln:
                            # gamma==1, beta==0: out = (v - mean) * rstd
                            nc.vector.tensor_scalar(
                                out=vout[:, :], in0=v[:, :],
                                scalar1=st[:, 0:1], scalar2=st[:, 2:3],
                                op0=mybir.AluOpType.subtract,
                                op1=mybir.AluOpType.mult,
                            )
                        else:
                            # out = ((v - mean)*gamma)*rstd + beta
                            nc.vector.scalar_tensor_tensor(
                                out=v[:, :], in0=v[:, :], scalar=st[:, 0:1],
                                in1=gam[:, :],
                                op0=mybir.AluOpType.subtract, op1=mybir.AluOpType.mult,
                            )
                            nc.vector.scalar_tensor_tensor(
                                out=vout[:, :], in0=v[:, :], scalar=st[:, 2:3],
                                in1=bet[:, :],
                                op0=mybir.AluOpType.mult, op1=mybir.AluOpType.add,
                            )
                        nc.sync.dma_start(out=out_d[128 * qp:128 * (qp + 1), :], in_=vout[:, :])
    nc.compile()
    return nc


_NC_CACHE = {}


def _pair_pack(a, np_chunks):
    """[np_chunks*256, N] -> [np_chunks*128, 2, N] DoubleRow pair layout."""
    n = a.shape[1]
    return np.ascontiguousarray(
        a.reshape(np_chunks, 2, 128, n).transpose(0, 2, 1, 3).reshape(np_chunks * 128, 2, n)
    )


def make_in_maps(inputs):
    x = np.asarray(inputs["inputs"], np.float32)
    wq = np.asarray(inputs["wq"], np.float32)
    wk = np.asarray(inputs["wk"], np.float32)
    wv = np.asarray(inputs["wv"], np.float32)
    wo = np.asarray(inputs["wo"], np.float32)
    bq = np.asarray(inputs["bq"], np.float32)
    f8 = lambda a: np.clip(a, -240.0, 240.0).astype(NPF8)
    bf = lambda a: np.ascontiguousarray(a).astype(ml_dtypes.bfloat16)
    bo_eff = np.asarray(inputs["bo"], np.float32) + np.asarray(inputs["bv"], np.float32) @ wo
    wqk = (wq @ wk.T) * WQK_S
    wvo = (wv @ wo) * WVO_S
    kcv = wk @ bq  # per-key logit offset direction: c[k] = x_k . kcv / 32
    shared = {
        "wqkp": f8(_pair_pack(wqk, NDP)),
        "wvop": f8(_pair_pack(wvo, NDP)),
        "gamma": bf(inputs["gamma"]),
        "beta": bf(inputs["beta"]),
    }
    in_maps = []
    for c in range(8):
        b, qh = c // 2, c % 2
        xb = x[b]                                  # [S, D]
        own = xb[Q * qh:Q * (qh + 1), :]
        oth = xb[Q * (1 - qh):Q * (2 - qh), :]
        xbp = np.concatenate([own, oth], axis=0)   # keys permuted: own half first
        cb = ((xbp @ kcv) / 32.0 - CSHIFT).reshape(NKC, 128).T
        in_maps.append({
            **shared,
            "xtp": f8(_pair_pack(xbp.T, NDP)),
            "xkp": f8(_pair_pack(xbp, NKP)),
            "xq": np.ascontiguousarray(own) + bo_eff[None, :],
            "cb": np.ascontiguousarray(cb),
        })
    return in_maps


def kernel(**inputs) -> np.ndarray:
    from concourse.bass_utils import run_bass_kernel_spmd

    fused_ln = bool(
        np.all(np.asarray(inputs["gamma"]) == 1.0)
        and np.all(np.asarray(inputs["beta"]) == 0.0)
    )
    if fused_ln not in _NC_CACHE:
        _NC_CACHE[fused_ln] = build_nc(fused_ln)
    res = run_bass_kernel_spmd(
        _NC_CACHE[fused_ln], make_in_maps(inputs), core_ids=list(range(8))
    )
    out = np.empty((B, S, D), np.float32)
    for c in range(8):
        b, qh = c // 2, c % 2
        out[b, Q * qh:Q * (qh + 1), :] = res.results[c]["out"].astype(np.float32)
    return out
